# revision 57
# baseline (speedup 1.0000x reference)
"""KAST scatter-memory kernel for Trainium2 (8 NeuronCores, data-parallel over batch).

Per core: one batch element, 15 sequential steps.

Host precomputes (numpy, inside kernel()):
  kt   = k transposed to [seq, ck, hw]   (no on-device PE transposes)
  g    = sigmoid(attention)              (no on-device sigmoid / ACT table swaps)
  v4   = [v | 1] ones-padded, partition-major, bf16  (denominator column ready)
  g4   = gate natural-layout replicated x4, partition-major, bf16

Device, per step i:
  m_kT = m_kT + G*(kT_i - m_kT)          (EMA: ck-chunk 0 on DVE, chunk 1 on
                                          Pool as 512-col halves; step 0 is
                                          just G*kT on DVE)
  L    = kT_i^T/m_kT^T @ kT_{i+1}        ([kk, q] fp32r matmuls, 4 per slot)
  E    = exp(L - 60) in bf16, split across 3 engines per sim:
           6/8 slots: ACT single-slot exps (1024-wide; pairs would
                      structurally stall the 4-slot psum rotation)
           2/8 slots: DVE  y = (L-60)*log2e  (psum read, 512-col halves),
                      Pool E = pow(2, y)     (gpsimd; exact, sbuf-only)
         step 0 runs all-ACT plans (DVE/Pool busy with prologue); the
         last step shifts two em slots to X so ACT drains sooner.
  rec  = E^T(stationary) @ [pv|1]/[mv|1] (tiny [128,4]-output matmuls; moving
         operand bf16 so fp32r's small-ap 4x/row penalty is avoided)
  rec  = 0.9*Nk/Dk + 0.1*Nm/Dm          (DVE reciprocal/blend)
  pv   = mask_i ? v_i : rec              (pv/mv state kept bf16)

PSUM is one persistent [128, 4, 1024] tile (all 8 banks) used as 4 rotating
fill slots; rec accumulators live in transient slot corners (subtile deps).
Engine budget per steady step (~15.3us): PE 14.1 (fills are the wall:
2 sims x 16384 fp32r rows/step at 1 cyc/row), ACT ~12.5, Pool ~12.9,
DVE ~10. Startup: PE warm-up matmuls from ~1us (Pool-made weights), kT0/kT1
quarter DMAs alternating across the SP and ACT HWDGE queues.
"""
import sys

sys.path.insert(0, "/opt/trn_rl_repo")

import numpy as np

import concourse.bass as bass
import concourse.tile as tile
from concourse import bacc, mybir
from concourse.bass_utils import run_bass_kernel_spmd

F32 = mybir.dt.float32
F32R = mybir.dt.float32r
BF16 = mybir.dt.bfloat16
AF = mybir.ActivationFunctionType
OP = mybir.AluOpType

BS, SEQ, H, W, CK = 8, 16, 32, 32, 256
HW = H * W          # 1024
CV = 3
NT = HW // 128      # 8 hw tiles
NC2 = CK // 128     # 2 ck chunks
SHIFT = 60.0        # exp(logit - SHIFT); logits empirically <= 136, rowmax >= 23
LOG2E = float(np.log2(np.e))
COEF = 0.1

# exp plan per sim: ("S", t) = ACT single-slot exp, ("X", t) = DVE scale/bias
# + Pool pow(2, ·), issued as two 512-col halves so the slot frees early.
# All-singles: a 2048-wide pair takes ~2.1us from ACT start but its first
# slot is needed ~1.7us after fill -- pairs structurally stall the 4-slot
# rotation; singles (1.3us) never do.
EK_PLAN = [("X", 0), ("X", 1)] + [("S", t) for t in range(2, 8)]
EM_PLAN = [("X", 0), ("X", 1)] + [("S", t) for t in range(2, 8)]
# step 0: ACT starts empty so the first pair fits the slot deadline; singles
# keep mid-sim latency low; late pairs get rec/em-section slack. DVE/Pool are
# busy with prologue DMAs + G0-gated EMA -- no X offload in step 0.
STEP0_PLAN = [("P", 0), ("S", 2), ("S", 3), ("P", 4), ("P", 6)]
EK_PLAN_LAST = EK_PLAN
EM_PLAN_LAST = (
    [("S", 0), ("S", 1), ("S", 2), ("S", 3), ("S", 4), ("X", 5), ("X", 6), ("S", 7)]
)

_CACHE = {}


def _r(x):
    return x.bitcast(F32R)


def build_program():
    nc = bacc.Bacc("TRN2", target_bir_lowering=False, debug=False, num_devices=8)

    kt_d = nc.dram_tensor("kt", [SEQ, CK, HW], F32R, kind="ExternalInput")
    v4_d = nc.dram_tensor("v4h", [128, SEQ, NT * 4], BF16, kind="ExternalInput")
    g_d = nc.dram_tensor("gflat", [1, SEQ * HW], BF16, kind="ExternalInput")
    g4_d = nc.dram_tensor("g4h", [128, SEQ, NT * 4], BF16, kind="ExternalInput")
    m_d = nc.dram_tensor("maskf", [1, SEQ], F32, kind="ExternalInput")
    o_d = nc.dram_tensor("out_v", [SEQ - 1, HW, CV], F32, kind="ExternalOutput")

    with tile.TileContext(nc) as tc:
        with (
            tc.tile_pool(name="persist", bufs=1) as P1,
            tc.tile_pool(name="kt", bufs=8) as PKT,
            tc.tile_pool(name="tmp", bufs=2) as PT,
            tc.tile_pool(name="gb", bufs=3) as PG,
            tc.tile_pool(name="ek", bufs=9) as PEK,
            tc.tile_pool(name="em", bufs=9) as PEM,
            tc.tile_pool(name="yx", bufs=4) as PY,
            tc.tile_pool(name="small", bufs=3) as PSM,
            tc.tile_pool(name="psA", bufs=1, space="PSUM") as PSA,
        ):
            negC = P1.tile([128, 1], F32)
            nc.vector.memset(negC, -SHIFT)
            two1 = P1.tile([128, 1], F32, tag="two")
            nc.vector.memset(two1, 2.0)
            two512 = two1.broadcast_to([128, 512])

            # All of PSUM: 4 rotating fill slots of [128, 1024] (2 banks each)
            ps_all = PSA.tile([128, 4, HW], F32, tag="psall")
            rot = [0]

            def load_kT_chunks(i):
                """kT frame as 2 SEPARATE tiles (one per ck chunk): dependency
                tracking is tile-granular, so c0-only matmuls never wait for
                the c1 DMA."""
                kT = []
                for c in range(NC2):
                    t = PKT.tile([128, HW], F32R, tag="kT", name=f"kT{i}c{c}")
                    nc.sync.dma_start(
                        out=t,
                        in_=kt_d[i, c * 128 : (c + 1) * 128, :].rearrange(
                            "(o p) w -> p o w", p=128
                        ),
                    )
                    kT.append(t)
                return tuple(kT)

            def load_G(i):
                Gt = PG.tile([128, HW], BF16, tag="G", name=f"G{i}")
                nc.sync.dma_start(
                    out=Gt, in_=g_d[0:1, i * HW : (i + 1) * HW].partition_broadcast(128)
                )
                return Gt

            # prologue: per-chunk tiles; rhs (kT1) and lhs (kT0) c0 chunks on
            # the SP queue, c1 chunks on the ACT queue -- the first two
            # matmuls of every slot need only the c0 tiles
            kT01 = [
                [
                    PKT.tile([128, HW], F32R, tag="kT", name=f"kT{i}c{c}")
                    for c in range(NC2)
                ]
                for i in range(2)
            ]
            for (i, c), eng in zip(
                ((1, 0), (1, 1), (0, 0), (0, 1)),
                (nc.sync, nc.scalar, nc.sync, nc.scalar),
            ):
                eng.dma_start(
                    out=kT01[i][c],
                    in_=kt_d[i, c * 128 : (c + 1) * 128, :].rearrange(
                        "(o p) w -> p o w", p=128
                    ),
                )
            kT_i = tuple(kT01[0])
            kT_n = tuple(kT01[1])
            G_rows = [load_G(0), load_G(1)]
            # PE ramp warm-up: tiny matmuls (64-wide) keep PE busy from ~0.5us
            # so the first real fills run at full clock; slot-3 corner is
            # reused by fill #4 much later (WAR via subtile deps)
            Wm0 = P1.tile([128, 64], F32, tag="Wm0")
            nc.gpsimd.memset(Wm0, 0.0)
            Wm = P1.tile([128, 64], F32R, tag="Wm")
            nc.gpsimd.tensor_copy(out=Wm, in_=Wm0)
            for _ in range(30):
                nc.tensor.matmul(
                    ps_all[0:64, 3, 960:1024], Wm[:, 0:64], Wm,
                    start=True, stop=True,
                )
            Vall = P1.tile([128, SEQ, NT * 4], BF16, tag="Vall")
            nc.sync.dma_start(out=Vall, in_=v4_d[:, :, :])
            G4all = P1.tile([128, SEQ, NT * 4], BF16, tag="G4all")
            nc.sync.dma_start(out=G4all, in_=g4_d[:, :, :])
            Mall = P1.tile([128, SEQ], F32, tag="Mall")
            nc.sync.dma_start(out=Mall, in_=m_d[0:1, :].partition_broadcast(128))
            pv1 = Vall[:, 0, :]

            # persistent state: m_kT ping-pong [128, (c w)] = [ck-part, 2 x hw]
            m_kT = [P1.tile([128, NC2 * HW], F32, tag=f"mkT{j}", name=f"mkT{j}") for j in range(2)]
            mv1 = P1.tile([128, 4 * NT], BF16, tag="mv1")
            nc.vector.memset(mv1, 0.0)
            nc.vector.memset(mv1[:, 3 : 4 * NT : 4], 1.0)

            def ema_mkT(i, kTfrm):
                # chunk 0 on DVE; chunk 1 on Pool (512-col halves so Pool pow
                # responses are never head-of-line blocked behind a 2us op)
                mcur, mnew = m_kT[i % 2], m_kT[(i + 1) % 2]
                for c in range(NC2):
                    base = c * HW
                    kc = kTfrm[c].bitcast(F32)
                    if i == 0:
                        # m starts at zero: m1 = G*kT, no memset/sub/add needed
                        sl = slice(base, base + HW)
                        if c == 0:
                            nc.vector.tensor_mul(_r(mnew[:, sl]), kc, G_rows[i])
                        else:
                            for h in range(2):
                                hs = slice(base + h * 512, base + (h + 1) * 512)
                                nc.gpsimd.tensor_mul(
                                    _r(mnew[:, hs]), kc[:, h * 512 : (h + 1) * 512],
                                    G_rows[i][:, h * 512 : (h + 1) * 512],
                                )
                        continue
                    if c == 0:
                        sl = slice(base, base + HW)
                        tmp = PT.tile([128, HW], F32, tag="tmpk0")
                        nc.vector.tensor_sub(tmp, kc, mcur[:, sl])
                        nc.vector.tensor_mul(tmp, tmp, G_rows[i])
                        nc.vector.tensor_add(_r(mnew[:, sl]), mcur[:, sl], tmp)
                    else:
                        tmp = PT.tile([128, HW], F32, tag="tmpk1")
                        for h in range(2):
                            hs = slice(base + h * 512, base + (h + 1) * 512)
                            ts = slice(h * 512, (h + 1) * 512)
                            gs = G_rows[i][:, h * 512 : (h + 1) * 512]
                            nc.gpsimd.tensor_sub(
                                tmp[:, ts], kc[:, ts], mcur[:, hs]
                            )
                            nc.gpsimd.tensor_mul(tmp[:, ts], tmp[:, ts], gs)
                            nc.gpsimd.tensor_add(_r(mnew[:, hs]), mcur[:, hs], tmp[:, ts])
                return mnew

            def fill_slot(lhs, kTn, t):
                """4 c-outer fp32r matmuls of tile t into the next psum slot.
                lhs/kTn are per-chunk (tile, tile) pairs."""
                s = rot[0] % 4
                rot[0] += 1
                for c in range(NC2):
                    for half in range(2):
                        nc.tensor.matmul(
                            ps_all[:, s, half * 512 : (half + 1) * 512],
                            _r(lhs[c][:, t * 128 : (t + 1) * 128]),
                            _r(kTn[c][:, half * 512 : (half + 1) * 512]),
                            start=(c == 0),
                            stop=(c == NC2 - 1),
                        )
                return s

            mnew = None
            for i in range(SEQ - 1):
                kT_n2 = load_kT_chunks(i + 2) if i + 2 <= SEQ - 1 else None
                if i + 2 <= SEQ - 2:
                    G_rows.append(load_G(i + 2))
                gb32 = G4all[:, i, :]

                # --- m_v EMA: mv1 += gb32 * (pv1 - mv1)  (ones col stays 1; bf16)
                tmpv = PSM.tile([128, 4 * NT], BF16, tag="tmpv")
                nc.vector.tensor_sub(tmpv, pv1, mv1)
                nc.vector.tensor_mul(tmpv, tmpv, gb32)
                nc.vector.tensor_add(mv1, mv1, tmpv)

                # --- logits + exps for both sims (fills first, recs later so
                # the next sim's fills are never queued behind rec matmuls)
                if i < 1:
                    ek_plan, em_plan = STEP0_PLAN, STEP0_PLAN
                elif i == SEQ - 2:
                    ek_plan, em_plan = EK_PLAN_LAST, EM_PLAN_LAST
                else:
                    ek_plan, em_plan = EK_PLAN, EM_PLAN
                for lhs_is_m, pool, nm, plan in (
                    (False, PEK, "ek", ek_plan),
                    (True, PEM, "em", em_plan),
                ):
                    if lhs_is_m and i == 0:
                        # issue step-0 EMA here so its G0-gated ops never
                        # head-of-line block the ek0 X-slot instr1s/pows
                        mnew = ema_mkT(0, kT_i)
                    lhs = (
                        (mnew[:, 0:HW], mnew[:, HW : 2 * HW]) if lhs_is_m else kT_i
                    )
                    E = [None] * NT   # per k-tile: (tile, base_offset)
                    for kind, t in plan:
                        if kind == "P":
                            s0 = fill_slot(lhs, kT_n, t)
                            s1 = fill_slot(lhs, kT_n, t + 1)
                            assert s1 == s0 + 1 and s0 % 2 == 0, (s0, s1)
                            Ep = pool.tile(
                                [128, 2 * HW], BF16, tag=nm, name=f"{nm}{i}_{t}"
                            )
                            nc.scalar.activation(
                                Ep,
                                ps_all[:, s0 : s0 + 2, :].rearrange("p s w -> p (s w)"),
                                AF.Exp,
                                bias=negC[:, 0:1],
                            )
                            E[t] = (Ep, 0)
                            E[t + 1] = (Ep, HW)
                        elif kind == "S":
                            s0 = fill_slot(lhs, kT_n, t)
                            Es = pool.tile(
                                [128, HW], BF16, tag=nm + "s", name=f"{nm}s{i}_{t}"
                            )
                            nc.scalar.activation(
                                Es, ps_all[:, s0, :], AF.Exp, bias=negC[:, 0:1]
                            )
                            E[t] = (Es, 0)
                        else:  # "X": DVE scale/bias from psum, Pool pow from sbuf
                            s0 = fill_slot(lhs, kT_n, t)
                            y = PY.tile([128, HW], F32, tag="y", name=f"y{nm}{i}_{t}")
                            Ex = pool.tile(
                                [128, HW], BF16, tag=nm + "s", name=f"{nm}x{i}_{t}"
                            )
                            for h in range(2):
                                hs = slice(h * 512, (h + 1) * 512)
                                nc.vector.tensor_scalar(
                                    out=y[:, hs], in0=ps_all[:, s0, hs],
                                    scalar1=LOG2E, scalar2=-SHIFT * LOG2E,
                                    op0=OP.mult, op1=OP.add,
                                )
                                nc.gpsimd.tensor_tensor(
                                    out=Ex[:, hs], in0=two512, in1=y[:, hs], op=OP.pow
                                )
                            E[t] = (Ex, 0)
                    assert all(e is not None for e in E)

                    # rec for this sim (moving operand bf16: 1 cyc/row)
                    rhs1 = pv1 if nm == "ek" else mv1
                    cs = (rot[0] + 3) % 4
                    base = 0 if nm == "ek" else 32
                    psN = ps_all[:, cs, base : base + 32]
                    # accumulate ACT-produced tiles first, offloaded (X) tiles
                    # last, so the in-order PE never waits on a late Pool pow
                    if i == SEQ - 2:
                        xset = {t for kind, t in plan if kind == "X"}
                        corder = [c for c in range(NT) if c not in xset] + sorted(xset)
                    else:
                        corder = list(range(NT))
                    for q in range(NT):
                        out_sl = psN[:, q * 4 : (q + 1) * 4]
                        for j, c in enumerate(corder):
                            Et, Eb = E[c]
                            nc.tensor.matmul(
                                out_sl,
                                Et[:, Eb + q * 128 : Eb + (q + 1) * 128],
                                rhs1[:, c * 4 : (c + 1) * 4],
                                start=(j == 0),
                                stop=(j == NT - 1),
                            )
                    Nhalf = PSM.tile([128, 32], F32, tag=f"N{nm}", name=f"N{nm}{i}")
                    nc.vector.tensor_copy(out=Nhalf, in_=psN)
                    coef = (1.0 - COEF) if nm == "ek" else COEF
                    rDh = PSM.tile([128, 8], F32, tag=f"rD{nm}")
                    nc.vector.reciprocal(rDh, Nhalf[:, 3:32:4])
                    rDeh = PSM.tile([128, 8, 4], F32, tag=f"rDe{nm}")
                    nc.vector.tensor_scalar_mul(
                        rDeh, rDh.unsqueeze(-1).broadcast_to([128, 8, 4]), coef
                    )
                    Nsh = PSM.tile([128, 32], F32, tag=f"Ns{nm}", name=f"Ns{nm}{i}")
                    nc.vector.tensor_mul(
                        Nsh, Nhalf, rDeh.rearrange("p t c -> p (t c)")
                    )
                    if nm == "ek":
                        Nsk = Nsh
                    else:
                        Nsm = Nsh

                # --- EMA for the next step, issued now so the DVE FIFO
                # completes it long before step i+1's E_m fills need it
                if i + 1 <= SEQ - 2:
                    mnext = ema_mkT(i + 1, kT_n)

                # --- rec = Nsk + Nsm
                rec = PSM.tile([128, 32], F32, tag="rec")
                nc.vector.tensor_add(rec, Nsk, Nsm)

                # --- write out_v[i] (pre-blend reconstruction)
                nc.sync.dma_start(
                    out=o_d[i].rearrange("(t p) c -> p t c", p=128),
                    in_=rec.rearrange("p (t c) -> p t c", c=4)[:, :, 0:CV],
                )

                # --- pv_next = rec + mask_i * (v1_i - rec)   (stored bf16)
                if i < SEQ - 2:
                    v1 = Vall[:, i, :]
                    diff = PSM.tile([128, 32], F32, tag="diff")
                    nc.vector.tensor_sub(diff, v1, rec)
                    nc.vector.tensor_scalar_mul(diff, diff, Mall[:, i : i + 1])
                    pvf = PSM.tile([128, 32], F32, tag="pvf")
                    nc.vector.tensor_add(pvf, rec, diff)
                    pv1_new = PSM.tile([128, 32], BF16, tag="pv1")
                    nc.vector.tensor_copy(out=pv1_new, in_=pvf)
                    pv1 = pv1_new
                    kT_i = kT_n
                    kT_n = kT_n2
                    mnew = mnext

    nc.compile()
    return nc


def _prep_inputs(k, v, attention, seq_mask):
    import ml_dtypes

    kt = np.ascontiguousarray(
        k.reshape(BS, SEQ, HW, CK).transpose(0, 1, 3, 2)
    )  # [bs, seq, ck, hw]
    v4 = np.ones((BS, SEQ, HW, 4), np.float32)
    v4[:, :, :, :CV] = v.reshape(BS, SEQ, HW, CV)
    # partition-major: [bs, 128, seq, 8*4]
    v4h = np.ascontiguousarray(
        v4.reshape(BS, SEQ, NT, 128, 4).transpose(0, 3, 1, 2, 4).reshape(
            BS, 128, SEQ, NT * 4
        ).astype(ml_dtypes.bfloat16)
    )
    g = (1.0 / (1.0 + np.exp(-attention.reshape(BS, SEQ, HW)))).astype(np.float32)
    gflat = np.ascontiguousarray(
        g.astype(ml_dtypes.bfloat16).reshape(BS, 1, SEQ * HW)
    )
    g4h = np.ascontiguousarray(
        np.repeat(
            g.reshape(BS, SEQ, NT, 128)[:, :, :, :, None], 4, axis=4
        ).transpose(0, 3, 1, 2, 4).reshape(BS, 128, SEQ, NT * 4).astype(
            ml_dtypes.bfloat16
        )
    )
    maskf = seq_mask.astype(np.float32)
    return kt, v4h, gflat, g4h, maskf


def kernel(k, v, attention, seq_mask):
    k = np.asarray(k, dtype=np.float32)
    v = np.asarray(v, dtype=np.float32)
    attention = np.asarray(attention, dtype=np.float32)
    seq_mask = np.asarray(seq_mask)

    if "nc" not in _CACHE:
        _CACHE["nc"] = build_program()
    nc = _CACHE["nc"]

    kt, v4h, gflat, g4h, maskf = _prep_inputs(k, v, attention, seq_mask)

    in_maps = []
    for b in range(BS):
        in_maps.append(
            {
                "kt": kt[b],
                "v4h": v4h[b],
                "gflat": gflat[b],
                "g4h": g4h[b],
                "maskf": np.ascontiguousarray(maskf[b : b + 1]),
            }
        )
    res = run_bass_kernel_spmd(nc, in_maps, list(range(BS)))
    out_v = np.stack([res.results[b]["out_v"] for b in range(BS)]).reshape(
        BS, SEQ - 1, H, W, CV
    )
    gt = v[:, 1:].reshape(BS, SEQ - 1, H, W, CV)
    return out_v, gt


# revision 58
# speedup vs baseline: 1.0059x; 1.0059x over previous
"""KAST scatter-memory kernel for Trainium2 (8 NeuronCores, data-parallel over batch).

Per core: one batch element, 15 sequential steps.

Host precomputes (numpy, inside kernel()):
  kt   = k transposed to [seq, ck, hw]   (no on-device PE transposes)
  g    = sigmoid(attention)              (no on-device sigmoid / ACT table swaps)
  v4   = [v | 1] ones-padded, partition-major, bf16  (denominator column ready)
  g4   = gate natural-layout replicated x4, partition-major, bf16

Device, per step i:
  m_kT = m_kT + G*(kT_i - m_kT)          (EMA: ck-chunk 0 on DVE, chunk 1 on
                                          Pool as 512-col halves; step 0 is
                                          just G*kT on DVE)
  L    = kT_i^T/m_kT^T @ kT_{i+1}        ([kk, q] fp32r matmuls, 4 per slot)
  E    = exp(L - 60) in bf16, split across 3 engines per sim:
           6/8 slots: ACT single-slot exps (1024-wide; pairs would
                      structurally stall the 4-slot psum rotation)
           2/8 slots: DVE  y = (L-60)*log2e  (psum read, 512-col halves),
                      Pool E = pow(2, y)     (gpsimd; exact, sbuf-only)
         step 0 runs all-ACT plans (DVE/Pool busy with prologue); the
         last step shifts two em slots to X so ACT drains sooner.
  rec  = E^T(stationary) @ [pv|1]/[mv|1] (tiny [128,4]-output matmuls; moving
         operand bf16 so fp32r's small-ap 4x/row penalty is avoided)
  rec  = 0.9*Nk/Dk + 0.1*Nm/Dm          (DVE reciprocal/blend)
  pv   = mask_i ? v_i : rec              (pv/mv state kept bf16)

PSUM is one persistent [128, 4, 1024] tile (all 8 banks) used as 4 rotating
fill slots; rec accumulators live in transient slot corners (subtile deps).
Engine budget per steady step (~15.3us): PE 14.1 (fills are the wall:
2 sims x 16384 fp32r rows/step at 1 cyc/row), ACT ~12.5, Pool ~12.9,
DVE ~10. Startup: PE warm-up matmuls from ~1us (Pool-made weights), kT0/kT1
quarter DMAs alternating across the SP and ACT HWDGE queues.
"""
import sys

sys.path.insert(0, "/opt/trn_rl_repo")

import numpy as np

import concourse.bass as bass
import concourse.tile as tile
from concourse import bacc, mybir
from concourse.bass_utils import run_bass_kernel_spmd

F32 = mybir.dt.float32
F32R = mybir.dt.float32r
BF16 = mybir.dt.bfloat16
AF = mybir.ActivationFunctionType
OP = mybir.AluOpType

BS, SEQ, H, W, CK = 8, 16, 32, 32, 256
HW = H * W          # 1024
CV = 3
NT = HW // 128      # 8 hw tiles
NC2 = CK // 128     # 2 ck chunks
SHIFT = 60.0        # exp(logit - SHIFT); logits empirically <= 136, rowmax >= 23
LOG2E = float(np.log2(np.e))
COEF = 0.1

# exp plan per sim: ("S", t) = ACT single-slot exp, ("X", t) = DVE scale/bias
# + Pool pow(2, ·), issued as two 512-col halves so the slot frees early.
# All-singles: a 2048-wide pair takes ~2.1us from ACT start but its first
# slot is needed ~1.7us after fill -- pairs structurally stall the 4-slot
# rotation; singles (1.3us) never do.
EK_PLAN = [("X", 0), ("X", 1)] + [("S", t) for t in range(2, 8)]
EM_PLAN = [("X", 0), ("X", 1)] + [("S", t) for t in range(2, 8)]
# step 0: ACT starts empty so the first pair fits the slot deadline; singles
# keep mid-sim latency low; late pairs get rec/em-section slack. DVE/Pool are
# busy with prologue DMAs + G0-gated EMA -- no X offload in step 0.
STEP0_PLAN = [("P", 0), ("S", 2), ("S", 3), ("P", 4), ("P", 6)]
EK_PLAN_LAST = EK_PLAN
EM_PLAN_LAST = (
    [("S", 0), ("S", 1), ("S", 2), ("S", 3), ("S", 4), ("X", 5), ("X", 6), ("S", 7)]
)

_CACHE = {}


def _r(x):
    return x.bitcast(F32R)


def build_program():
    nc = bacc.Bacc("TRN2", target_bir_lowering=False, debug=False, num_devices=8)

    kt_d = nc.dram_tensor("kt", [SEQ, CK, HW], F32R, kind="ExternalInput")
    v4_d = nc.dram_tensor("v4h", [128, SEQ, NT * 4], BF16, kind="ExternalInput")
    g_d = nc.dram_tensor("gflat", [1, SEQ * HW], BF16, kind="ExternalInput")
    g4_d = nc.dram_tensor("g4h", [128, SEQ, NT * 4], BF16, kind="ExternalInput")
    m_d = nc.dram_tensor("maskf", [1, SEQ], F32, kind="ExternalInput")
    o_d = nc.dram_tensor("out_v", [SEQ - 1, HW, CV], F32, kind="ExternalOutput")

    with tile.TileContext(nc) as tc:
        with (
            tc.tile_pool(name="persist", bufs=1) as P1,
            tc.tile_pool(name="kt", bufs=4) as PKT,
            tc.tile_pool(name="tmp", bufs=2) as PT,
            tc.tile_pool(name="gb", bufs=3) as PG,
            tc.tile_pool(name="ek", bufs=9) as PEK,
            tc.tile_pool(name="em", bufs=9) as PEM,
            tc.tile_pool(name="yx", bufs=4) as PY,
            tc.tile_pool(name="small", bufs=3) as PSM,
            tc.tile_pool(name="psA", bufs=1, space="PSUM") as PSA,
        ):
            negC = P1.tile([128, 1], F32)
            nc.vector.memset(negC, -SHIFT)
            two1 = P1.tile([128, 1], F32, tag="two")
            nc.vector.memset(two1, 2.0)
            two512 = two1.broadcast_to([128, 512])

            # All of PSUM: 4 rotating fill slots of [128, 1024] (2 banks each)
            ps_all = PSA.tile([128, 4, HW], F32, tag="psall")
            rot = [0]

            def load_kT_chunks(i):
                """kT frame as 2 chunk DMAs so first-chunk matmuls start early."""
                kT = PKT.tile([128, NC2, HW], F32R, tag="kT", name=f"kT{i}")
                for c in range(NC2):
                    nc.sync.dma_start(
                        out=kT[:, c, :],
                        in_=kt_d[i, c * 128 : (c + 1) * 128, :].rearrange(
                            "(o p) w -> p o w", p=128
                        ),
                    )
                return kT.rearrange("p c w -> p (c w)")

            def load_G(i):
                Gt = PG.tile([128, HW], BF16, tag="G", name=f"G{i}")
                nc.sync.dma_start(
                    out=Gt, in_=g_d[0:1, i * HW : (i + 1) * HW].partition_broadcast(128)
                )
                return Gt

            # prologue: kT frames first, c0 chunks of BOTH frames before c1
            # chunks (first matmuls need only c0), then everything bulky
            kT01 = [
                PKT.tile([128, NC2, HW], F32R, tag="kT", name=f"kT{i}")
                for i in range(2)
            ]
            # quarter-granular loads ordered so tiles 0-3 (lhs slices in the
            # first 512 cols) are fillable after six of the eight quarters
            for (i, c, h), eng in zip(
                (
                    (0, 0, 0), (1, 0, 0), (1, 0, 1), (0, 1, 0),
                    (1, 1, 0), (1, 1, 1), (0, 0, 1), (0, 1, 1),
                ),
                (nc.sync, nc.scalar, nc.sync, nc.scalar,
                 nc.sync, nc.scalar, nc.sync, nc.scalar),
            ):
                eng.dma_start(
                    out=kT01[i][:, c, h * 512 : (h + 1) * 512],
                    in_=kt_d[
                        i, c * 128 : (c + 1) * 128, h * 512 : (h + 1) * 512
                    ].rearrange("(o p) w -> p o w", p=128),
                )
            kT_i = kT01[0].rearrange("p c w -> p (c w)")
            kT_n = kT01[1].rearrange("p c w -> p (c w)")
            G_rows = [load_G(0), load_G(1)]
            # PE ramp warm-up: tiny matmuls (64-wide) keep PE busy from ~0.5us
            # so the first real fills run at full clock; slot-3 corner is
            # reused by fill #4 much later (WAR via subtile deps)
            Wm0 = P1.tile([128, 64], F32, tag="Wm0")
            nc.gpsimd.memset(Wm0, 0.0)
            Wm = P1.tile([128, 64], F32R, tag="Wm")
            nc.gpsimd.tensor_copy(out=Wm, in_=Wm0)
            for _ in range(30):
                nc.tensor.matmul(
                    ps_all[0:64, 3, 960:1024], Wm[:, 0:64], Wm,
                    start=True, stop=True,
                )
            Vall = P1.tile([128, SEQ, NT * 4], BF16, tag="Vall")
            nc.sync.dma_start(out=Vall, in_=v4_d[:, :, :])
            G4all = P1.tile([128, SEQ, NT * 4], BF16, tag="G4all")
            nc.sync.dma_start(out=G4all, in_=g4_d[:, :, :])
            Mall = P1.tile([128, SEQ], F32, tag="Mall")
            nc.sync.dma_start(out=Mall, in_=m_d[0:1, :].partition_broadcast(128))
            pv1 = Vall[:, 0, :]

            # persistent state: m_kT ping-pong [128, (c w)] = [ck-part, 2 x hw]
            m_kT = [P1.tile([128, NC2 * HW], F32, tag=f"mkT{j}", name=f"mkT{j}") for j in range(2)]
            mv1 = P1.tile([128, 4 * NT], BF16, tag="mv1")
            nc.vector.memset(mv1, 0.0)
            nc.vector.memset(mv1[:, 3 : 4 * NT : 4], 1.0)

            def ema_mkT(i, kTfrm):
                # chunk 0 on DVE; chunk 1 on Pool (512-col halves so Pool pow
                # responses are never head-of-line blocked behind a 2us op)
                mcur, mnew = m_kT[i % 2], m_kT[(i + 1) % 2]
                for c in range(NC2):
                    base = c * HW
                    if i == 0:
                        # m starts at zero: m1 = G*kT, no memset/sub/add needed
                        sl = slice(base, base + HW)
                        if c == 0:
                            nc.vector.tensor_mul(
                                _r(mnew[:, sl]), kTfrm[:, sl].bitcast(F32), G_rows[i]
                            )
                        else:
                            for h in range(2):
                                hs = slice(base + h * 512, base + (h + 1) * 512)
                                nc.gpsimd.tensor_mul(
                                    _r(mnew[:, hs]), kTfrm[:, hs].bitcast(F32),
                                    G_rows[i][:, h * 512 : (h + 1) * 512],
                                )
                        continue
                    if c == 0:
                        sl = slice(base, base + HW)
                        tmp = PT.tile([128, HW], F32, tag="tmpk0")
                        nc.vector.tensor_sub(tmp, kTfrm[:, sl].bitcast(F32), mcur[:, sl])
                        nc.vector.tensor_mul(tmp, tmp, G_rows[i])
                        nc.vector.tensor_add(_r(mnew[:, sl]), mcur[:, sl], tmp)
                    else:
                        tmp = PT.tile([128, HW], F32, tag="tmpk1")
                        for h in range(2):
                            hs = slice(base + h * 512, base + (h + 1) * 512)
                            ts = slice(h * 512, (h + 1) * 512)
                            gs = G_rows[i][:, h * 512 : (h + 1) * 512]
                            nc.gpsimd.tensor_sub(
                                tmp[:, ts], kTfrm[:, hs].bitcast(F32), mcur[:, hs]
                            )
                            nc.gpsimd.tensor_mul(tmp[:, ts], tmp[:, ts], gs)
                            nc.gpsimd.tensor_add(_r(mnew[:, hs]), mcur[:, hs], tmp[:, ts])
                return mnew

            def fill_slot(lhs, kTn, t):
                """4 c-outer fp32r matmuls of tile t into the next psum slot."""
                s = rot[0] % 4
                rot[0] += 1
                for c in range(NC2):
                    for half in range(2):
                        nc.tensor.matmul(
                            ps_all[:, s, half * 512 : (half + 1) * 512],
                            _r(lhs[:, c * HW + t * 128 : c * HW + (t + 1) * 128]),
                            _r(kTn[:, c * HW + half * 512 : c * HW + (half + 1) * 512]),
                            start=(c == 0),
                            stop=(c == NC2 - 1),
                        )
                return s

            mnew = None
            for i in range(SEQ - 1):
                kT_n2 = load_kT_chunks(i + 2) if i + 2 <= SEQ - 1 else None
                if i + 2 <= SEQ - 2:
                    G_rows.append(load_G(i + 2))
                gb32 = G4all[:, i, :]

                # --- m_v EMA: mv1 += gb32 * (pv1 - mv1)  (ones col stays 1; bf16)
                tmpv = PSM.tile([128, 4 * NT], BF16, tag="tmpv")
                nc.vector.tensor_sub(tmpv, pv1, mv1)
                nc.vector.tensor_mul(tmpv, tmpv, gb32)
                nc.vector.tensor_add(mv1, mv1, tmpv)

                # --- logits + exps for both sims (fills first, recs later so
                # the next sim's fills are never queued behind rec matmuls)
                if i < 1:
                    ek_plan, em_plan = STEP0_PLAN, STEP0_PLAN
                elif i == SEQ - 2:
                    ek_plan, em_plan = EK_PLAN_LAST, EM_PLAN_LAST
                else:
                    ek_plan, em_plan = EK_PLAN, EM_PLAN
                for lhs_is_m, pool, nm, plan in (
                    (False, PEK, "ek", ek_plan),
                    (True, PEM, "em", em_plan),
                ):
                    if lhs_is_m and i == 0:
                        # issue step-0 EMA here so its G0-gated ops never
                        # head-of-line block the ek0 X-slot instr1s/pows
                        mnew = ema_mkT(0, kT_i)
                    lhs = mnew if lhs_is_m else kT_i
                    E = [None] * NT   # per k-tile: (tile, base_offset)
                    for kind, t in plan:
                        if kind == "P":
                            s0 = fill_slot(lhs, kT_n, t)
                            s1 = fill_slot(lhs, kT_n, t + 1)
                            assert s1 == s0 + 1 and s0 % 2 == 0, (s0, s1)
                            Ep = pool.tile(
                                [128, 2 * HW], BF16, tag=nm, name=f"{nm}{i}_{t}"
                            )
                            nc.scalar.activation(
                                Ep,
                                ps_all[:, s0 : s0 + 2, :].rearrange("p s w -> p (s w)"),
                                AF.Exp,
                                bias=negC[:, 0:1],
                            )
                            E[t] = (Ep, 0)
                            E[t + 1] = (Ep, HW)
                        elif kind == "S":
                            s0 = fill_slot(lhs, kT_n, t)
                            Es = pool.tile(
                                [128, HW], BF16, tag=nm + "s", name=f"{nm}s{i}_{t}"
                            )
                            nc.scalar.activation(
                                Es, ps_all[:, s0, :], AF.Exp, bias=negC[:, 0:1]
                            )
                            E[t] = (Es, 0)
                        else:  # "X": DVE scale/bias from psum, Pool pow from sbuf
                            s0 = fill_slot(lhs, kT_n, t)
                            y = PY.tile([128, HW], F32, tag="y", name=f"y{nm}{i}_{t}")
                            Ex = pool.tile(
                                [128, HW], BF16, tag=nm + "s", name=f"{nm}x{i}_{t}"
                            )
                            for h in range(2):
                                hs = slice(h * 512, (h + 1) * 512)
                                nc.vector.tensor_scalar(
                                    out=y[:, hs], in0=ps_all[:, s0, hs],
                                    scalar1=LOG2E, scalar2=-SHIFT * LOG2E,
                                    op0=OP.mult, op1=OP.add,
                                )
                                nc.gpsimd.tensor_tensor(
                                    out=Ex[:, hs], in0=two512, in1=y[:, hs], op=OP.pow
                                )
                            E[t] = (Ex, 0)
                    assert all(e is not None for e in E)

                    # rec for this sim (moving operand bf16: 1 cyc/row)
                    rhs1 = pv1 if nm == "ek" else mv1
                    cs = (rot[0] + 3) % 4
                    base = 0 if nm == "ek" else 32
                    psN = ps_all[:, cs, base : base + 32]
                    # accumulate ACT-produced tiles first, offloaded (X) tiles
                    # last, so the in-order PE never waits on a late Pool pow
                    if i == SEQ - 2:
                        xset = {t for kind, t in plan if kind == "X"}
                        corder = [c for c in range(NT) if c not in xset] + sorted(xset)
                    else:
                        corder = list(range(NT))
                    for q in range(NT):
                        out_sl = psN[:, q * 4 : (q + 1) * 4]
                        for j, c in enumerate(corder):
                            Et, Eb = E[c]
                            nc.tensor.matmul(
                                out_sl,
                                Et[:, Eb + q * 128 : Eb + (q + 1) * 128],
                                rhs1[:, c * 4 : (c + 1) * 4],
                                start=(j == 0),
                                stop=(j == NT - 1),
                            )
                    Nhalf = PSM.tile([128, 32], F32, tag=f"N{nm}", name=f"N{nm}{i}")
                    nc.vector.tensor_copy(out=Nhalf, in_=psN)
                    coef = (1.0 - COEF) if nm == "ek" else COEF
                    rDh = PSM.tile([128, 8], F32, tag=f"rD{nm}")
                    nc.vector.reciprocal(rDh, Nhalf[:, 3:32:4])
                    rDeh = PSM.tile([128, 8, 4], F32, tag=f"rDe{nm}")
                    nc.vector.tensor_scalar_mul(
                        rDeh, rDh.unsqueeze(-1).broadcast_to([128, 8, 4]), coef
                    )
                    Nsh = PSM.tile([128, 32], F32, tag=f"Ns{nm}", name=f"Ns{nm}{i}")
                    nc.vector.tensor_mul(
                        Nsh, Nhalf, rDeh.rearrange("p t c -> p (t c)")
                    )
                    if nm == "ek":
                        Nsk = Nsh
                    else:
                        Nsm = Nsh

                # --- EMA for the next step, issued now so the DVE FIFO
                # completes it long before step i+1's E_m fills need it
                if i + 1 <= SEQ - 2:
                    mnext = ema_mkT(i + 1, kT_n)

                # --- rec = Nsk + Nsm
                rec = PSM.tile([128, 32], F32, tag="rec")
                nc.vector.tensor_add(rec, Nsk, Nsm)

                # --- write out_v[i] (pre-blend reconstruction)
                nc.sync.dma_start(
                    out=o_d[i].rearrange("(t p) c -> p t c", p=128),
                    in_=rec.rearrange("p (t c) -> p t c", c=4)[:, :, 0:CV],
                )

                # --- pv_next = rec + mask_i * (v1_i - rec)   (stored bf16)
                if i < SEQ - 2:
                    v1 = Vall[:, i, :]
                    diff = PSM.tile([128, 32], F32, tag="diff")
                    nc.vector.tensor_sub(diff, v1, rec)
                    nc.vector.tensor_scalar_mul(diff, diff, Mall[:, i : i + 1])
                    pvf = PSM.tile([128, 32], F32, tag="pvf")
                    nc.vector.tensor_add(pvf, rec, diff)
                    pv1_new = PSM.tile([128, 32], BF16, tag="pv1")
                    nc.vector.tensor_copy(out=pv1_new, in_=pvf)
                    pv1 = pv1_new
                    kT_i = kT_n
                    kT_n = kT_n2
                    mnew = mnext

    nc.compile()
    return nc


def _prep_inputs(k, v, attention, seq_mask):
    import ml_dtypes

    kt = np.ascontiguousarray(
        k.reshape(BS, SEQ, HW, CK).transpose(0, 1, 3, 2)
    )  # [bs, seq, ck, hw]
    v4 = np.ones((BS, SEQ, HW, 4), np.float32)
    v4[:, :, :, :CV] = v.reshape(BS, SEQ, HW, CV)
    # partition-major: [bs, 128, seq, 8*4]
    v4h = np.ascontiguousarray(
        v4.reshape(BS, SEQ, NT, 128, 4).transpose(0, 3, 1, 2, 4).reshape(
            BS, 128, SEQ, NT * 4
        ).astype(ml_dtypes.bfloat16)
    )
    g = (1.0 / (1.0 + np.exp(-attention.reshape(BS, SEQ, HW)))).astype(np.float32)
    gflat = np.ascontiguousarray(
        g.astype(ml_dtypes.bfloat16).reshape(BS, 1, SEQ * HW)
    )
    g4h = np.ascontiguousarray(
        np.repeat(
            g.reshape(BS, SEQ, NT, 128)[:, :, :, :, None], 4, axis=4
        ).transpose(0, 3, 1, 2, 4).reshape(BS, 128, SEQ, NT * 4).astype(
            ml_dtypes.bfloat16
        )
    )
    maskf = seq_mask.astype(np.float32)
    return kt, v4h, gflat, g4h, maskf


def kernel(k, v, attention, seq_mask):
    k = np.asarray(k, dtype=np.float32)
    v = np.asarray(v, dtype=np.float32)
    attention = np.asarray(attention, dtype=np.float32)
    seq_mask = np.asarray(seq_mask)

    if "nc" not in _CACHE:
        _CACHE["nc"] = build_program()
    nc = _CACHE["nc"]

    kt, v4h, gflat, g4h, maskf = _prep_inputs(k, v, attention, seq_mask)

    in_maps = []
    for b in range(BS):
        in_maps.append(
            {
                "kt": kt[b],
                "v4h": v4h[b],
                "gflat": gflat[b],
                "g4h": g4h[b],
                "maskf": np.ascontiguousarray(maskf[b : b + 1]),
            }
        )
    res = run_bass_kernel_spmd(nc, in_maps, list(range(BS)))
    out_v = np.stack([res.results[b]["out_v"] for b in range(BS)]).reshape(
        BS, SEQ - 1, H, W, CV
    )
    gt = v[:, 1:].reshape(BS, SEQ - 1, H, W, CV)
    return out_v, gt
